# revision 32
# baseline (speedup 1.0000x reference)
"""FADE upsampling kernel for 8 Trainium2 NeuronCores.

Shards: core = 2*b + half  (b in 0..4 batches, half = top/bottom 32 lo-res rows).
Each core computes out[b, :, 64*half : 64*half+64, :]  (256 ch, 64 hi rows, 128 cols).

Host pre-pads halos with zeros so the device kernel has no edge cases:
  en shard: (2,128,66,130)  = en[b, :, 2r0-1:2r0+65, :] padded, cols padded +-1
  de shard: (2,128,36,68)   = de[b, :, r0-2:r0+34, :] padded, cols padded +-2

Device pipeline (per core), channels-on-partitions layout:
  1) lo-res: gate conv1x1 (+sigmoid both ways), dec conv1x1 (partition-dup,
     row-shifted lower half), dec 3x3 conv -> logits_de, de->bf16,
     PE transposes -> depT[w', s, a, c], DMA dup -> dg_pair (rows s, s+1
     stacked on partitions for K=72 matmuls)
  2) hi-res blocks (8 x 8 rows): enc conv1x1 (dup+shift), 3x3 conv via
     bf16 tap-pairs -> logits psum, + up2(logits_de), exp (ACT),
     sum over taps (PE ones-matmul), A = (1-g)/sum (DVE divide with
     DMA-broadcast (1-g)), ker_scaled = exp*A -> bf16 -> DMA to ker_dram
  3) scatter ker_dram into banded S in DRAM (bands are contiguous
     128-element runs thanks to (u, pp, x, v) column order), read back,
     carafe = 3 banded matmuls per (pair, half, chunk) with K=72/72/36,
     combine out = g*en + psum, DMA out.

S-matrix math: out[c, o, j] = sum_{dy,dx} ker[(dy,dx), o, j] *
  dep[c, o//2+dy, j//2+dx]  (shard coords; pads are zero).
For half h (j in [64h, 64h+64)): w' = j//2+dx-32h = u+dx where u=jj//2.
S groups: A=dy{0,1} via dg_pair[h][:, pp], B=dy{2,3} via dg_pair[h][:, pp+2],
C=dy{4} via dg_pair[h][0:36, pp+4].
"""

import numpy as np
import ml_dtypes

import concourse.bass as bass
import concourse.tile as tile
import concourse.mybir as mybir
from concourse import bacc

F32 = mybir.dt.float32
BF16 = mybir.dt.bfloat16

B, C, H, W = 4, 256, 64, 64          # de dims; en is (B,C,2H,2W)
EMB = 64
NA = 2                               # channel chunks of 128
P = 128
LO_R, LO_C = 36, 68                  # de shard with halo 2
EN_R, EN_C = 66, 130                 # en shard with halo 1
OR_, OC = 64, 128                    # out rows / cols per core
NPAIR = 32                           # output row pairs
SROW = 5 * 36 * 2                    # 360 rows in DRAM S buffer (A,B,C x h)
SCOL = 32 * NPAIR * 2 * 2            # 4096 cols: (u, pp, x, v)

_NC_CACHE = {}


def _build_nc():
    nc = bacc.Bacc("TRN2")

    en_d = nc.dram_tensor("en", [NA, P, EN_R, EN_C], F32, kind="ExternalInput")
    de_d = nc.dram_tensor("de", [NA, P, LO_R, LO_C], F32, kind="ExternalInput")
    wg_d = nc.dram_tensor("wg", [P, NA], F32, kind="ExternalInput")
    wgb_d = nc.dram_tensor("wgb", [1, 1], F32, kind="ExternalInput")
    wgbn_d = nc.dram_tensor("wgbn", [1, 1], F32, kind="ExternalInput")
    wcen_d = nc.dram_tensor("wcen", [P, NA, P], F32, kind="ExternalInput")
    bcen_d = nc.dram_tensor("bcen", [P, 1], F32, kind="ExternalInput")
    wcde_d = nc.dram_tensor("wcde", [P, NA, P], F32, kind="ExternalInput")
    wce2_d = nc.dram_tensor("wce2", [P, 3, 25], BF16, kind="ExternalInput")
    wce1_d = nc.dram_tensor("wce1", [EMB, 3, 25], BF16, kind="ExternalInput")
    bce_d = nc.dram_tensor("bce", [25, 1], F32, kind="ExternalInput")
    id_d = nc.dram_tensor("ident", [P, P], BF16, kind="ExternalInput")
    # pre-zeroed S scratch; bands are overwritten on device every run
    s_dram = nc.dram_tensor("s_zero", [SROW * SCOL], BF16,
                            kind="ExternalInput")
    out_d = nc.dram_tensor("out", [NA, P, OR_, OC], F32, kind="ExternalOutput")

    g_scr = nc.dram_tensor("g_scratch", [32 * 64], F32)
    t1_scr = nc.dram_tensor("t1_scratch", [32 * 64], F32)

    with tile.TileContext(nc) as tc:
        _body(tc, nc, en_d, de_d, wg_d, wgb_d, wgbn_d, wcen_d, bcen_d,
              wcde_d, wce2_d, wce1_d, bce_d, id_d, out_d, s_dram,
              g_scr, t1_scr)
    nc.compile()
    return nc


def _body(tc, nc, en_d, de_d, wg_d, wgb_d, wgbn_d, wcen_d, bcen_d,
          wcde_d, wce2_d, wce1_d, bce_d, id_d, out_d, s_dram,
          g_scr, t1_scr):
    from contextlib import ExitStack

    with ExitStack() as ctx:
        const = ctx.enter_context(tc.tile_pool(name="const", bufs=1))
        big = ctx.enter_context(tc.tile_pool(name="big", bufs=1))

        # ---- constants / weights ----
        ident = const.tile([P, P], BF16)
        nc.sync.dma_start(ident, id_d[:, :])
        ones25 = const.tile([25, 25], F32)
        nc.vector.memset(ones25, 1.0)

        wg_sb = const.tile([P, NA], F32)
        nc.sync.dma_start(wg_sb, wg_d[:, :])
        wgb_sb = const.tile([1, 1], F32)
        nc.sync.dma_start(wgb_sb, wgb_d[:, :])
        wgbn_sb = const.tile([1, 1], F32)
        nc.sync.dma_start(wgbn_sb, wgbn_d[:, :])
        wcen_sb = const.tile([P, NA, P], F32)
        nc.sync.dma_start(wcen_sb, wcen_d[:, :, :])
        bcen_sb = const.tile([P, 1], F32)
        nc.sync.dma_start(bcen_sb, bcen_d[:, :])
        wcde_sb = const.tile([P, NA, P], F32)
        nc.sync.dma_start(wcde_sb, wcde_d[:, :, :])
        wce2_sb = const.tile([P, 3, 25], BF16)
        nc.sync.dma_start(wce2_sb, wce2_d[:, :, :])
        wce1_sb = const.tile([EMB, 3, 25], BF16)
        nc.sync.dma_start(wce1_sb, wce1_d[:, :, :])
        bce_sb = const.tile([25, 1], F32)
        nc.sync.dma_start(bce_sb, bce_d[:, :])

        # ---- persistent buffers ----
        en_sb = big.tile([P, NA, EN_R, EN_C], F32)
        for a in range(NA):
            nc.sync.dma_start(en_sb[:, a], en_d[a])
        # dg_pair[h]: partitions (d in {0,1}) x 36 w'rel; free (sb, a, c):
        # dg_pair[h][36d+w'', sb, a, c] = depT[32h+w'', sb+d, a, c]
        dg_pair = [big.tile([72, LO_R, NA, P], BF16, name=f"dgp_{h}")
                   for h in range(2)]

        mid_pool = tc.tile_pool(name="mid", bufs=1)
        mid = mid_pool.__enter__()
        g_lo = mid.tile([1, 32 * 64], F32)
        t1g_lo = mid.tile([1, 32 * 64], F32)
        t1g_b = mid.tile([25, 32 * 64], F32)
        ld_sb = mid.tile([25, 32 * 64], F32)
        # ker_scaled, SBUF cols per tap: (h, u, pp, x, v)
        ker_s = mid.tile([25, 2 * 4096], BF16)

        s1bp_pool = tc.tile_pool(name="s1bp", bufs=1)
        s1bp = s1bp_pool.__enter__()
        de_bf = s1bp.tile([P, NA, LO_R, LO_C], BF16)
        depT = s1bp.tile([LO_C, LO_R, NA, P], BF16)

        s1a_pool = tc.tile_pool(name="s1a", bufs=1)
        s1b = s1a_pool.__enter__()
        de_sb = s1b.tile([P, NA, LO_R, LO_C], F32)
        for a in range(NA):
            nc.sync.dma_start(de_sb[:, a], de_d[a])
        dec2 = s1b.tile([P, LO_R * LO_C], BF16)

        # ---- stage 1: lo-res ----
        with tc.tile_pool(name="ps_s1", bufs=2, space="PSUM") as ps_s1, \
             tc.tile_pool(name="ps_g", bufs=2, space="PSUM") as ps_g, \
             tc.tile_pool(name="ps_t", bufs=2, space="PSUM") as ps_t:

            # gate: sigmoid(w_gate . de + b), and 1-sigmoid
            for nb in range(4):
                s0 = 2 + nb * 8
                ps = ps_g.tile([1, 512], F32)
                for a in range(NA):
                    rhs = de_sb[:, a, s0:s0 + 8, 2:66]
                    nc.tensor.matmul(ps, wg_sb[:, a:a + 1], rhs,
                                     start=(a == 0), stop=(a == 1))
                nc.scalar.activation(g_lo[:, nb * 512:(nb + 1) * 512], ps,
                                     mybir.ActivationFunctionType.Sigmoid,
                                     bias=wgb_sb, scale=1.0)
                nc.scalar.activation(t1g_lo[:, nb * 512:(nb + 1) * 512], ps,
                                     mybir.ActivationFunctionType.Sigmoid,
                                     bias=wgbn_sb, scale=-1.0)
            # replicate gate maps across partitions via DMA broadcast
            # (bounce through DRAM: SBUF sources cannot have step-0
            # partition dims)
            nc.sync.dma_start(t1_scr[:].unsqueeze(0), t1g_lo[0:1, :])
            nc.sync.dma_start(g_scr[:].unsqueeze(0), g_lo[0:1, :])
            nc.sync.dma_start(
                t1g_b, bass.AP(tensor=t1_scr, offset=0,
                               ap=[[0, 25], [1, 32 * 64]]))

            # dec = w_cde . de (no bias), dup on partitions, bf16,
            # lower half shifted one lo row (dec2[64:, s, :] = dec[s+1])
            NDE = LO_R * LO_C
            de_flat = de_sb.rearrange("p a r c -> p a (r c)")
            for nb in range(5):
                n0 = nb * 512
                n1 = min(NDE, n0 + 512)
                nn = n1 - n0
                ps = ps_s1.tile([P, 512], F32)
                for a in range(NA):
                    nc.tensor.matmul(ps[:, :nn], wcde_sb[:, a],
                                     de_flat[:, a, n0:n1],
                                     start=(a == 0), stop=(a == 1))
                nc.scalar.activation(dec2[0:EMB, n0:n1], ps[0:EMB, :nn],
                                     mybir.ActivationFunctionType.Copy)
                lo = max(0, n0 - LO_C)
                hi = n1 - LO_C
                if hi > lo:
                    so = lo + LO_C - n0
                    nc.vector.tensor_copy(dec2[EMB:P, lo:hi],
                                          ps[EMB:P, so:so + (hi - lo)])

            # logits_de = 3x3 conv over dec (+ b_ce), valid rows s in [2,34)
            dec2_v = dec2.rearrange("p (r c) -> p r c", c=LO_C)
            for nb in range(4):
                s0 = 2 + nb * 8
                ps = ps_s1.tile([25, 512], F32)
                for dx in range(3):
                    rhs = dec2_v[:, s0 - 1:s0 + 7, 1 + dx:65 + dx]
                    nc.tensor.matmul(ps, wce2_sb[:, dx], rhs,
                                     start=(dx == 0), stop=False)
                for dx in range(3):
                    rhs = dec2_v[0:EMB, s0 + 1:s0 + 9, 1 + dx:65 + dx]
                    nc.tensor.matmul(ps, wce1_sb[:, dx], rhs,
                                     start=False, stop=(dx == 2))
                nc.scalar.activation(ld_sb[:, nb * 512:(nb + 1) * 512], ps,
                                     mybir.ActivationFunctionType.Identity,
                                     bias=bce_sb, scale=1.0)

            # de -> bf16, then PE transposes into depT[w', s, a, c]
            nc.vector.tensor_copy(
                de_bf.rearrange("p a r c -> p (a r c)"),
                de_sb.rearrange("p a r c -> p (a r c)"))
            for s in range(LO_R):
                for a in range(NA):
                    pst = ps_t.tile([LO_C, P], BF16)
                    nc.tensor.transpose(pst, de_bf[:, a, s, :], ident)
                    if (s + a) % 2 == 0:
                        nc.vector.tensor_copy(depT[:, s, a, :], pst)
                    else:
                        nc.scalar.activation(
                            depT[:, s, a, :], pst,
                            mybir.ActivationFunctionType.Copy)

        s1a_pool.__exit__(None, None, None)
        # row-pair duplicated depT (d=1 slice only valid for sb<35)
        for h in range(2):
            nc.sync.dma_start(dg_pair[h][0:36], depT[32 * h:32 * h + 36])
            nc.sync.dma_start(dg_pair[h][36:72, 0:LO_R - 1],
                              depT[32 * h:32 * h + 36, 1:LO_R])
        s1bp_pool.__exit__(None, None, None)

        # ---- stage 2: hi-res blocks ----
        ker_v = ker_s.rearrange("t (h u q x v) -> t h u q x v",
                                h=2, u=32, q=NPAIR, x=2)
        with tc.tile_pool(name="enc2p", bufs=2) as enc2p, \
             tc.tile_pool(name="work2", bufs=3) as work2, \
             tc.tile_pool(name="ps_enc", bufs=3, space="PSUM") as ps_enc, \
             tc.tile_pool(name="ps_lg", bufs=2, space="PSUM") as ps_lg, \
             tc.tile_pool(name="ps_sum", bufs=2, space="PSUM") as ps_sum:

            en_flat = en_sb.rearrange("p a r c -> p a (r c)")
            ld_v = ld_sb.rearrange("p (r c) -> p r c", c=64)
            t1g_bv = t1g_b.rearrange("p (r c) -> p r c", c=64)
            for blk in range(8):
                t0 = blk * 8
                NEN = 10 * EN_C  # 1300
                enc2 = enc2p.tile([P, NEN], BF16)
                for nb in range(3):
                    n0 = nb * 512
                    n1 = min(NEN, n0 + 512)
                    nn = n1 - n0
                    ps = ps_enc.tile([P, 512], F32)
                    for a in range(NA):
                        nc.tensor.matmul(
                            ps[:, :nn], wcen_sb[:, a],
                            en_flat[:, a, t0 * EN_C + n0:t0 * EN_C + n1],
                            start=(a == 0), stop=(a == 1))
                    nc.scalar.activation(enc2[0:EMB, n0:n1], ps[0:EMB, :nn],
                                         mybir.ActivationFunctionType.Identity,
                                         bias=bcen_sb[0:EMB], scale=1.0)
                    lo = max(0, n0 - EN_C)
                    hi = min(NEN - EN_C, n1 - EN_C)
                    if hi > lo:
                        so = lo + EN_C - n0
                        nc.vector.tensor_scalar(
                            out=enc2[EMB:P, lo:hi],
                            in0=ps[EMB:P, so:so + (hi - lo)],
                            scalar1=bcen_sb[EMB:P], scalar2=None,
                            op0=mybir.AluOpType.add)

                enc2_v = enc2.rearrange("p (r c) -> p r c", c=EN_C)
                for nb2 in range(2):
                    u0 = nb2 * 4
                    ps = ps_lg.tile([25, 512], F32)
                    for dx in range(3):
                        rhs = enc2_v[:, u0:u0 + 4, dx:dx + OC]
                        nc.tensor.matmul(ps, wce2_sb[:, dx], rhs,
                                         start=(dx == 0), stop=False)
                    for dx in range(3):
                        rhs = enc2_v[0:EMB, u0 + 2:u0 + 6, dx:dx + OC]
                        nc.tensor.matmul(ps, wce1_sb[:, dx], rhs,
                                         start=False, stop=(dx == 2))

                    lr0 = blk * 4 + nb2 * 2
                    # up2-add of logits_de, split by (row parity b, col
                    # parity d) so every AP is partition + <=2 free dims
                    ein = work2.tile([25, 512], F32)
                    ein_v = ein.rearrange("p (a b c d) -> p a b c d",
                                          a=2, b=2, c=64)
                    ps_v = ps.rearrange("p (a b c d) -> p a b c d",
                                        a=2, b=2, c=64)
                    for bb in range(2):
                        for dd in range(2):
                            nc.vector.tensor_tensor(
                                out=ein_v[:, :, bb, :, dd],
                                in0=ps_v[:, :, bb, :, dd],
                                in1=ld_v[:, lr0:lr0 + 2, :],
                                op=mybir.AluOpType.add)
                    es = work2.tile([25, 512], F32)
                    nc.scalar.activation(es, ein,
                                         mybir.ActivationFunctionType.Exp,
                                         bias=bce_sb, scale=1.0)
                    sm = ps_sum.tile([25, 512], F32)
                    nc.tensor.matmul(sm, ones25, es, start=True, stop=True)
                    rc = work2.tile([25, 512], F32)
                    nc.vector.reciprocal(rc, sm)
                    aa = work2.tile([25, 512], F32)
                    aa_v = aa.rearrange("p (a b c d) -> p a b c d",
                                        a=2, b=2, c=64)
                    rc_v = rc.rearrange("p (a b c d) -> p a b c d",
                                        a=2, b=2, c=64)
                    for bb in range(2):
                        for dd in range(2):
                            nc.vector.tensor_tensor(
                                out=aa_v[:, :, bb, :, dd],
                                in0=t1g_bv[:, lr0:lr0 + 2, :],
                                in1=rc_v[:, :, bb, :, dd],
                                op=mybir.AluOpType.mult)
                    # ker_scaled = es * aa -> bf16 directly into ker_s
                    # (cols (h, u, pp, x, v)); split per (q, x, h) so every
                    # AP is partition + 2 free dims in matching (u, v) order
                    pp0 = blk * 4 + nb2 * 2
                    es_v = es.rearrange("p (q x h u v) -> p q x h u v",
                                        q=2, x=2, h=2, u=32)
                    aa_v5 = aa.rearrange("p (q x h u v) -> p q x h u v",
                                         q=2, x=2, h=2, u=32)
                    for q in range(2):
                        for xx in range(2):
                            for h in range(2):
                                nc.vector.tensor_tensor(
                                    out=ker_v[:, h, :, pp0 + q, xx, :],
                                    in0=es_v[:, q, xx, h],
                                    in1=aa_v5[:, q, xx, h],
                                    op=mybir.AluOpType.mult)

        # ---- scatter ker into banded S (DRAM -> DRAM) ----
        # S row (group-relative) = 36*(dy - dy0g) + u + dx; bands for fixed
        # (dy,dx,h,u) are 128 contiguous elements (pp,x,v).
        for dy in range(5):
            g, dy0g = (0, 0) if dy < 2 else ((1, 2) if dy < 4 else (2, 4))
            gbase = [0, 4 * 36, 8 * 36][g]
            grows = 72 if g < 2 else 36
            for dx in range(5):
                tau = dy * 5 + dx
                for h in range(2):
                    base = (gbase + h * grows + 36 * (dy - dy0g) + dx) * SCOL
                    # dst walks (u, (pp,x), v); src is contiguous
                    dst = bass.AP(tensor=s_dram, offset=base,
                                  ap=[[SCOL + 2, 32], [64, 64], [1, 2]])
                    nc.sync.dma_start(
                        dst, ker_s[tau:tau + 1, h * 4096:(h + 1) * 4096])

        # read back
        mid_pool.__exit__(None, None, None)
        s3big = ctx.enter_context(tc.tile_pool(name="s3big", bufs=1))
        s_sbA = [s3big.tile([72, SCOL], BF16, name=f"sA_{h}") for h in range(2)]
        s_sbB = [s3big.tile([72, SCOL], BF16, name=f"sB_{h}") for h in range(2)]
        s_sbC = [s3big.tile([36, SCOL], BF16, name=f"sC_{h}") for h in range(2)]
        s_dram_v = s_dram[:].rearrange("(r c) -> r c", c=SCOL)
        for h in range(2):
            nc.sync.dma_start(s_sbA[h], s_dram_v[h * 72:(h + 1) * 72])
            nc.sync.dma_start(s_sbB[h],
                              s_dram_v[144 + h * 72:144 + (h + 1) * 72])
            nc.sync.dma_start(s_sbC[h],
                              s_dram_v[288 + h * 36:288 + (h + 1) * 36])
        # hi-res gate map g_hi[p, pp, j] = g(pp, j//2), via 2 broadcast DMAs
        g_hi = s3big.tile([P, NPAIR, OC], F32)
        g_hi_v = g_hi.rearrange("p q (w v) -> p q w v", v=2)
        for vv in range(2):
            nc.sync.dma_start(
                g_hi_v[:, :, :, vv],
                bass.AP(tensor=g_scr, offset=0,
                        ap=[[0, P], [64, NPAIR], [1, 64]]))

        # ---- stage 3: carafe + combine ----
        with tc.tile_pool(name="work3", bufs=4) as work3, \
             tc.tile_pool(name="outp", bufs=2) as outp, \
             tc.tile_pool(name="ps_c", bufs=4, space="PSUM") as ps_c:

            # S cols are (pp, x, u, v)
            sA_v = [s_sbA[h].rearrange("k (q x u v) -> k q x u v",
                                       q=NPAIR, x=2, u=32) for h in range(2)]
            sB_v = [s_sbB[h].rearrange("k (q x u v) -> k q x u v",
                                       q=NPAIR, x=2, u=32) for h in range(2)]
            sC_v = [s_sbC[h].rearrange("k (q x u v) -> k q x u v",
                                       q=NPAIR, x=2, u=32) for h in range(2)]
            for grp in range(8):  # groups of 4 pairs
                ost = [outp.tile([P, 8, OC], F32, name=f"ost{a}",
                                 tag=f"ost{a}") for a in range(NA)]
                for lp in range(4):
                    pp = grp * 4 + lp
                    for a in range(NA):
                        # psum free: (h, x, u, v)
                        ps = ps_c.tile([P, 2, 2, 32, 2], F32)
                        for h in range(2):
                            out_ap = ps[:, h]
                            nc.tensor.matmul(out_ap,
                                             dg_pair[h][:, pp, a, :],
                                             sA_v[h][:, pp],
                                             start=True, stop=False)
                            nc.tensor.matmul(out_ap,
                                             dg_pair[h][:, pp + 2, a, :],
                                             sB_v[h][:, pp],
                                             start=False, stop=False)
                            nc.tensor.matmul(out_ap,
                                             dg_pair[h][0:36, pp + 4, a, :],
                                             sC_v[h][:, pp],
                                             start=False, stop=True)
                        for xx in range(2):
                            tm = work3.tile([P, OC], F32, tag="tm")
                            en_r = en_sb[:, a, 2 * pp + 1 + xx, 1:129]
                            nc.vector.tensor_tensor(
                                out=tm, in0=en_r, in1=g_hi[:, pp, :],
                                op=mybir.AluOpType.mult)
                            orow = (ost[a][:, 2 * lp + xx, :]
                                    .rearrange("p (h w) -> p h w", h=2))
                            nc.vector.tensor_tensor(
                                out=orow,
                                in0=tm.rearrange("p (h w) -> p h w", h=2),
                                in1=ps[:, :, xx].rearrange(
                                    "p h u v -> p h (u v)"),
                                op=mybir.AluOpType.add)
                for a in range(NA):
                    nc.sync.dma_start(out_d[a, :, grp * 8:grp * 8 + 8, :],
                                      ost[a])


def _get_nc():
    if "nc" not in _NC_CACHE:
        _NC_CACHE["nc"] = _build_nc()
    return _NC_CACHE["nc"]


def _shard_inputs(en, de, w_gate, b_gate, w_cen, b_cen, w_cde, w_ce, b_ce):
    en = np.asarray(en, np.float32)
    de = np.asarray(de, np.float32)
    wgT = np.asarray(w_gate, np.float32).reshape(C)          # (256,)
    wcen = np.asarray(w_cen, np.float32).reshape(EMB, C)
    wcde = np.asarray(w_cde, np.float32).reshape(EMB, C)
    wce = np.asarray(w_ce, np.float32)                       # (25,64,3,3)
    b_gate = np.asarray(b_gate, np.float32).reshape(1)
    b_cen = np.asarray(b_cen, np.float32).reshape(EMB)
    b_ce = np.asarray(b_ce, np.float32).reshape(25)

    wg_h = wgT.reshape(NA, P).T.copy()                        # [128,2]
    wcen_h = np.zeros((P, NA, P), np.float32)
    wcde_h = np.zeros((P, NA, P), np.float32)
    for a in range(NA):
        wt = wcen[:, a * P:(a + 1) * P].T                     # [128,64]
        wcen_h[:, a, 0:EMB] = wt
        wcen_h[:, a, EMB:P] = wt
        wt2 = wcde[:, a * P:(a + 1) * P].T
        wcde_h[:, a, 0:EMB] = wt2
        wcde_h[:, a, EMB:P] = wt2
    bcen_h = np.concatenate([b_cen, b_cen]).reshape(P, 1).astype(np.float32)

    wce2_h = np.zeros((P, 3, 25), np.float32)
    wce1_h = np.zeros((EMB, 3, 25), np.float32)
    for dx in range(3):
        wce2_h[0:EMB, dx, :] = wce[:, :, 0, dx].T
        wce2_h[EMB:P, dx, :] = wce[:, :, 1, dx].T
        wce1_h[:, dx, :] = wce[:, :, 2, dx].T
    wce2_h = wce2_h.astype(ml_dtypes.bfloat16)
    wce1_h = wce1_h.astype(ml_dtypes.bfloat16)

    base = {
        "wg": wg_h, "wgb": b_gate.reshape(1, 1),
        "wgbn": (-b_gate).reshape(1, 1).astype(np.float32),
        "wcen": wcen_h, "bcen": bcen_h, "wcde": wcde_h,
        "wce2": wce2_h, "wce1": wce1_h,
        "bce": b_ce.reshape(25, 1),
        "ident": np.eye(P, dtype=np.float32).astype(ml_dtypes.bfloat16),
        "s_zero": np.zeros(SROW * SCOL, dtype=ml_dtypes.bfloat16),
    }

    in_maps = []
    for core in range(8):
        b, half = divmod(core, 2)
        r0 = half * 32
        # en shard rows 2r0-1 .. 2r0+65, cols padded +-1
        en_sh = np.zeros((C, EN_R, EN_C), np.float32)
        lo = 2 * r0 - 1
        hi = 2 * r0 + 65
        slo, shi = max(0, lo), min(2 * H, hi)
        en_sh[:, slo - lo:shi - lo, 1:129] = en[b, :, slo:shi, :]
        # de shard rows r0-2 .. r0+34, cols padded +-2
        de_sh = np.zeros((C, LO_R, LO_C), np.float32)
        dlo, dhi = r0 - 2, r0 + 34
        sdlo, sdhi = max(0, dlo), min(H, dhi)
        de_sh[:, sdlo - dlo:sdhi - dlo, 2:66] = de[b, :, sdlo:sdhi, :]
        m = dict(base)
        m["en"] = en_sh.reshape(NA, P, EN_R, EN_C)
        m["de"] = de_sh.reshape(NA, P, LO_R, LO_C)
        in_maps.append(m)
    return in_maps


def kernel(**inputs):
    from concourse.bass_utils import run_bass_kernel_spmd
    nc = _get_nc()
    in_maps = _shard_inputs(**inputs)
    res = run_bass_kernel_spmd(nc, in_maps, core_ids=list(range(8)))
    out = np.zeros((B, C, 2 * H, 2 * W), np.float32)
    for core in range(8):
        b, half = divmod(core, 2)
        o = res.results[core]["out"]                # [2,128,64,128]
        out[b, :, 64 * half:64 * half + 64, :] = o.reshape(C, OR_, OC)
    return out


# revision 33
# speedup vs baseline: 2.7569x; 2.7569x over previous
"""FADE upsampling kernel for 8 Trainium2 NeuronCores.

Shards: core = 2*b + half  (b in 0..4 batches, half = top/bottom 32 lo-res rows).
Each core computes out[b, :, 64*half : 64*half+64, :]  (256 ch, 64 hi rows, 128 cols).

Host pre-pads halos with zeros so the device kernel has no edge cases:
  en shard: (2,128,66,130)  = en[b, :, 2r0-1:2r0+65, :] padded, cols padded +-1
  de shard: (2,128,36,68)   = de[b, :, r0-2:r0+34, :] padded, cols padded +-2

Device pipeline (per core), channels-on-partitions layout:
  1) lo-res: gate conv1x1 (+sigmoid both ways), dec conv1x1 (partition-dup,
     row-shifted lower half), dec 3x3 conv -> logits_de, de->bf16,
     PE transposes -> depT[w', s, a, c], DMA dup -> dg_pair (rows s, s+1
     stacked on partitions for K=72 matmuls)
  2) hi-res blocks (8 x 8 rows): enc conv1x1 (dup+shift), 3x3 conv via
     bf16 tap-pairs -> logits psum, + up2(logits_de), exp (ACT),
     sum over taps (PE ones-matmul), A = (1-g)/sum (DVE divide with
     DMA-broadcast (1-g)), ker_scaled = exp*A -> bf16 -> DMA to ker_dram
  3) scatter ker_dram into banded S in DRAM (bands are contiguous
     128-element runs thanks to (u, pp, x, v) column order), read back,
     carafe = 3 banded matmuls per (pair, half, chunk) with K=72/72/36,
     combine out = g*en + psum, DMA out.

S-matrix math: out[c, o, j] = sum_{dy,dx} ker[(dy,dx), o, j] *
  dep[c, o//2+dy, j//2+dx]  (shard coords; pads are zero).
For half h (j in [64h, 64h+64)): w' = j//2+dx-32h = u+dx where u=jj//2.
S groups: A=dy{0,1} via dg_pair[h][:, pp], B=dy{2,3} via dg_pair[h][:, pp+2],
C=dy{4} via dg_pair[h][0:36, pp+4].
"""

import numpy as np
import ml_dtypes

import concourse.bass as bass
import concourse.tile as tile
import concourse.mybir as mybir
from concourse import bacc

F32 = mybir.dt.float32
BF16 = mybir.dt.bfloat16

B, C, H, W = 4, 256, 64, 64          # de dims; en is (B,C,2H,2W)
EMB = 64
NA = 2                               # channel chunks of 128
P = 128
LO_R, LO_C = 36, 68                  # de shard with halo 2
EN_R, EN_C = 66, 130                 # en shard with halo 1
OR_, OC = 64, 128                    # out rows / cols per core
NPAIR = 32                           # output row pairs
SROW = 5 * 36 * 2                    # 360 rows in DRAM S buffer (A,B,C x h)
SCOL = 32 * NPAIR * 2 * 2            # 4096 cols: (u, pp, x, v)

_NC_CACHE = {}


def _build_nc():
    nc = bacc.Bacc("TRN2")

    en_d = nc.dram_tensor("en", [NA, P, EN_R, EN_C], F32, kind="ExternalInput")
    de_d = nc.dram_tensor("de", [NA, P, LO_R, LO_C], F32, kind="ExternalInput")
    wg_d = nc.dram_tensor("wg", [P, NA], F32, kind="ExternalInput")
    wgb_d = nc.dram_tensor("wgb", [1, 1], F32, kind="ExternalInput")
    wgbn_d = nc.dram_tensor("wgbn", [1, 1], F32, kind="ExternalInput")
    wcen_d = nc.dram_tensor("wcen", [P, NA, P], F32, kind="ExternalInput")
    bcen_d = nc.dram_tensor("bcen", [P, 1], F32, kind="ExternalInput")
    wcde_d = nc.dram_tensor("wcde", [P, NA, P], F32, kind="ExternalInput")
    wce2_d = nc.dram_tensor("wce2", [P, 3, 25], BF16, kind="ExternalInput")
    wce1_d = nc.dram_tensor("wce1", [EMB, 3, 25], BF16, kind="ExternalInput")
    bce_d = nc.dram_tensor("bce", [25, 1], F32, kind="ExternalInput")
    id_d = nc.dram_tensor("ident", [P, P], BF16, kind="ExternalInput")
    # pre-zeroed S scratch; bands are overwritten on device every run
    sA_d = nc.dram_tensor("s_zeroA", [144 * SCOL], BF16, kind="ExternalInput")
    sB_d = nc.dram_tensor("s_zeroB", [144 * SCOL], BF16, kind="ExternalInput")
    sC_d = nc.dram_tensor("s_zeroC", [72 * SCOL], BF16, kind="ExternalInput")
    out_d = nc.dram_tensor("out", [NA, P, OR_, OC], F32, kind="ExternalOutput")

    g_scr = nc.dram_tensor("g_scratch", [32 * 64], F32)
    t1_scr = nc.dram_tensor("t1_scratch", [32 * 64], F32)

    with tile.TileContext(nc) as tc:
        _body(tc, nc, en_d, de_d, wg_d, wgb_d, wgbn_d, wcen_d, bcen_d,
              wcde_d, wce2_d, wce1_d, bce_d, id_d, out_d,
              (sA_d, sB_d, sC_d), g_scr, t1_scr)
    nc.compile()
    return nc


def _body(tc, nc, en_d, de_d, wg_d, wgb_d, wgbn_d, wcen_d, bcen_d,
          wcde_d, wce2_d, wce1_d, bce_d, id_d, out_d, s_drams,
          g_scr, t1_scr):
    sA_d, sB_d, sC_d = s_drams
    from contextlib import ExitStack

    with ExitStack() as ctx:
        const = ctx.enter_context(tc.tile_pool(name="const", bufs=1))
        big = ctx.enter_context(tc.tile_pool(name="big", bufs=1))

        # ---- constants / weights ----
        ident = const.tile([P, P], BF16)
        nc.sync.dma_start(ident, id_d[:, :])
        ones25 = const.tile([25, 25], F32)
        nc.vector.memset(ones25, 1.0)

        wg_sb = const.tile([P, NA], F32)
        nc.sync.dma_start(wg_sb, wg_d[:, :])
        wgb_sb = const.tile([1, 1], F32)
        nc.sync.dma_start(wgb_sb, wgb_d[:, :])
        wgbn_sb = const.tile([1, 1], F32)
        nc.sync.dma_start(wgbn_sb, wgbn_d[:, :])
        wcen_sb = const.tile([P, NA, P], F32)
        nc.sync.dma_start(wcen_sb, wcen_d[:, :, :])
        bcen_sb = const.tile([P, 1], F32)
        nc.sync.dma_start(bcen_sb, bcen_d[:, :])
        wcde_sb = const.tile([P, NA, P], F32)
        nc.sync.dma_start(wcde_sb, wcde_d[:, :, :])
        wce2_sb = const.tile([P, 3, 25], BF16)
        nc.sync.dma_start(wce2_sb, wce2_d[:, :, :])
        wce1_sb = const.tile([EMB, 3, 25], BF16)
        nc.sync.dma_start(wce1_sb, wce1_d[:, :, :])
        bce_sb = const.tile([25, 1], F32)
        nc.sync.dma_start(bce_sb, bce_d[:, :])

        # ---- persistent buffers ----
        en_sb = big.tile([P, NA, EN_R, EN_C], F32)
        nc.sync.dma_start(en_sb[:, 0], en_d[0])
        nc.scalar.dma_start(en_sb[:, 1], en_d[1])
        # dg_pair[h]: partitions (d in {0,1}) x 36 w'rel; free (sb, a, c):
        # dg_pair[h][36d+w'', sb, a, c] = depT[32h+w'', sb+d, a, c]
        dg_pair = [big.tile([72, LO_R, NA, P], BF16, name=f"dgp_{h}")
                   for h in range(2)]

        mid_pool = tc.tile_pool(name="mid", bufs=1)
        mid = mid_pool.__enter__()
        g_lo = mid.tile([1, 32 * 64], F32)
        t1g_lo = mid.tile([1, 32 * 64], F32)
        t1g_b = mid.tile([25, 32 * 64], F32)
        ld_sb = mid.tile([25, 32 * 64], F32)
        # ker_scaled, SBUF cols per tap: (h, u, pp, x, v)
        ker_s = mid.tile([25, 2 * 4096], BF16)

        s1bp_pool = tc.tile_pool(name="s1bp", bufs=1)
        s1bp = s1bp_pool.__enter__()
        de_bf = s1bp.tile([P, NA, LO_R, LO_C], BF16)
        depT = s1bp.tile([LO_C, LO_R, NA, P], BF16)

        s1a_pool = tc.tile_pool(name="s1a", bufs=1)
        s1b = s1a_pool.__enter__()
        de_sb = s1b.tile([P, NA, LO_R, LO_C], F32)
        nc.sync.dma_start(de_sb[:, 0], de_d[0])
        nc.scalar.dma_start(de_sb[:, 1], de_d[1])
        dec2 = s1b.tile([P, LO_R * LO_C], BF16)

        # ---- stage 1: lo-res ----
        with tc.tile_pool(name="ps_s1", bufs=2, space="PSUM") as ps_s1, \
             tc.tile_pool(name="ps_g", bufs=2, space="PSUM") as ps_g, \
             tc.tile_pool(name="ps_t", bufs=2, space="PSUM") as ps_t:

            # gate: sigmoid(w_gate . de + b), and 1-sigmoid
            for nb in range(4):
                s0 = 2 + nb * 8
                ps = ps_g.tile([1, 512], F32)
                for a in range(NA):
                    rhs = de_sb[:, a, s0:s0 + 8, 2:66]
                    nc.tensor.matmul(ps, wg_sb[:, a:a + 1], rhs,
                                     start=(a == 0), stop=(a == 1))
                nc.scalar.activation(g_lo[:, nb * 512:(nb + 1) * 512], ps,
                                     mybir.ActivationFunctionType.Sigmoid,
                                     bias=wgb_sb, scale=1.0)
                nc.scalar.activation(t1g_lo[:, nb * 512:(nb + 1) * 512], ps,
                                     mybir.ActivationFunctionType.Sigmoid,
                                     bias=wgbn_sb, scale=-1.0)
            # replicate gate maps across partitions via DMA broadcast
            # (bounce through DRAM: SBUF sources cannot have step-0
            # partition dims)
            nc.sync.dma_start(t1_scr[:].unsqueeze(0), t1g_lo[0:1, :])
            nc.sync.dma_start(g_scr[:].unsqueeze(0), g_lo[0:1, :])
            nc.sync.dma_start(
                t1g_b, bass.AP(tensor=t1_scr, offset=0,
                               ap=[[0, 25], [1, 32 * 64]]))

            # dec = w_cde . de (no bias), dup on partitions, bf16,
            # lower half shifted one lo row (dec2[64:, s, :] = dec[s+1])
            NDE = LO_R * LO_C
            de_flat = de_sb.rearrange("p a r c -> p a (r c)")
            for nb in range(5):
                n0 = nb * 512
                n1 = min(NDE, n0 + 512)
                nn = n1 - n0
                ps = ps_s1.tile([P, 512], F32)
                for a in range(NA):
                    nc.tensor.matmul(ps[:, :nn], wcde_sb[:, a],
                                     de_flat[:, a, n0:n1],
                                     start=(a == 0), stop=(a == 1))
                nc.scalar.activation(dec2[0:EMB, n0:n1], ps[0:EMB, :nn],
                                     mybir.ActivationFunctionType.Copy)
                lo = max(0, n0 - LO_C)
                hi = n1 - LO_C
                if hi > lo:
                    so = lo + LO_C - n0
                    nc.vector.tensor_copy(dec2[EMB:P, lo:hi],
                                          ps[EMB:P, so:so + (hi - lo)])

            # logits_de = 3x3 conv over dec (+ b_ce), valid rows s in [2,34)
            dec2_v = dec2.rearrange("p (r c) -> p r c", c=LO_C)
            for nb in range(4):
                s0 = 2 + nb * 8
                ps = ps_s1.tile([25, 512], F32)
                for dx in range(3):
                    rhs = dec2_v[:, s0 - 1:s0 + 7, 1 + dx:65 + dx]
                    nc.tensor.matmul(ps, wce2_sb[:, dx], rhs,
                                     start=(dx == 0), stop=False)
                for dx in range(3):
                    rhs = dec2_v[0:EMB, s0 + 1:s0 + 9, 1 + dx:65 + dx]
                    nc.tensor.matmul(ps, wce1_sb[:, dx], rhs,
                                     start=False, stop=(dx == 2))
                nc.scalar.activation(ld_sb[:, nb * 512:(nb + 1) * 512], ps,
                                     mybir.ActivationFunctionType.Identity,
                                     bias=bce_sb, scale=1.0)

            # de -> bf16, then PE transposes into depT[w', s, a, c]
            nc.vector.tensor_copy(
                de_bf.rearrange("p a r c -> p (a r c)"),
                de_sb.rearrange("p a r c -> p (a r c)"))
            for s in range(LO_R):
                for a in range(NA):
                    pst = ps_t.tile([LO_C, P], BF16)
                    nc.tensor.transpose(pst, de_bf[:, a, s, :], ident)
                    if (s + a) % 2 == 0:
                        nc.vector.tensor_copy(depT[:, s, a, :], pst)
                    else:
                        nc.scalar.activation(
                            depT[:, s, a, :], pst,
                            mybir.ActivationFunctionType.Copy)

        s1a_pool.__exit__(None, None, None)
        # row-pair duplicated depT (d=1 slice only valid for sb<35)
        for h in range(2):
            nc.scalar.dma_start(dg_pair[h][0:36], depT[32 * h:32 * h + 36])
            nc.scalar.dma_start(dg_pair[h][36:72, 0:LO_R - 1],
                                depT[32 * h:32 * h + 36, 1:LO_R])
        s1bp_pool.__exit__(None, None, None)

        # ---- stage 2: hi-res blocks ----
        ker_v = ker_s.rearrange("t (h u x v q) -> t h u x v q",
                                h=2, u=32, x=2, v=2)
        with tc.tile_pool(name="enc2p", bufs=2) as enc2p, \
             tc.tile_pool(name="work2", bufs=3) as work2, \
             tc.tile_pool(name="ps_enc", bufs=3, space="PSUM") as ps_enc, \
             tc.tile_pool(name="ps_lg", bufs=2, space="PSUM") as ps_lg, \
             tc.tile_pool(name="ps_sum", bufs=2, space="PSUM") as ps_sum:

            en_flat = en_sb.rearrange("p a r c -> p a (r c)")
            ld_v = ld_sb.rearrange("p (r c) -> p r c", c=64)
            t1g_bv = t1g_b.rearrange("p (r c) -> p r c", c=64)
            for blk in range(8):
                t0 = blk * 8
                NEN = 10 * EN_C  # 1300
                enc2 = enc2p.tile([P, NEN], BF16)
                for nb in range(3):
                    n0 = nb * 512
                    n1 = min(NEN, n0 + 512)
                    nn = n1 - n0
                    ps = ps_enc.tile([P, 512], F32)
                    for a in range(NA):
                        nc.tensor.matmul(
                            ps[:, :nn], wcen_sb[:, a],
                            en_flat[:, a, t0 * EN_C + n0:t0 * EN_C + n1],
                            start=(a == 0), stop=(a == 1))
                    nc.scalar.activation(enc2[0:EMB, n0:n1], ps[0:EMB, :nn],
                                         mybir.ActivationFunctionType.Identity,
                                         bias=bcen_sb[0:EMB], scale=1.0)
                    lo = max(0, n0 - EN_C)
                    hi = min(NEN - EN_C, n1 - EN_C)
                    if hi > lo:
                        so = lo + EN_C - n0
                        nc.vector.tensor_scalar(
                            out=enc2[EMB:P, lo:hi],
                            in0=ps[EMB:P, so:so + (hi - lo)],
                            scalar1=bcen_sb[EMB:P], scalar2=None,
                            op0=mybir.AluOpType.add)

                enc2_v = enc2.rearrange("p (r c) -> p r c", c=EN_C)
                for nb2 in range(2):
                    u0 = nb2 * 4
                    ps = ps_lg.tile([25, 512], F32)
                    for dx in range(3):
                        rhs = enc2_v[:, u0:u0 + 4, dx:dx + OC]
                        nc.tensor.matmul(ps, wce2_sb[:, dx], rhs,
                                         start=(dx == 0), stop=False)
                    for dx in range(3):
                        rhs = enc2_v[0:EMB, u0 + 2:u0 + 6, dx:dx + OC]
                        nc.tensor.matmul(ps, wce1_sb[:, dx], rhs,
                                         start=False, stop=(dx == 2))

                    lr0 = blk * 4 + nb2 * 2
                    # up2-add of logits_de, split by (row parity b, col
                    # parity d) so every AP is partition + <=2 free dims
                    ein = work2.tile([25, 512], F32)
                    ein_v = ein.rearrange("p (a b c d) -> p a b c d",
                                          a=2, b=2, c=64)
                    ps_v = ps.rearrange("p (a b c d) -> p a b c d",
                                        a=2, b=2, c=64)
                    for bb in range(2):
                        for dd in range(2):
                            nc.vector.tensor_tensor(
                                out=ein_v[:, :, bb, :, dd],
                                in0=ps_v[:, :, bb, :, dd],
                                in1=ld_v[:, lr0:lr0 + 2, :],
                                op=mybir.AluOpType.add)
                    es = work2.tile([25, 512], F32)
                    nc.scalar.activation(es, ein,
                                         mybir.ActivationFunctionType.Exp,
                                         bias=bce_sb, scale=1.0)
                    sm = ps_sum.tile([25, 512], F32)
                    nc.tensor.matmul(sm, ones25, es, start=True, stop=True)
                    rc = work2.tile([25, 512], F32)
                    nc.vector.reciprocal(rc, sm)
                    aa = work2.tile([25, 512], F32)
                    aa_v = aa.rearrange("p (a b c d) -> p a b c d",
                                        a=2, b=2, c=64)
                    rc_v = rc.rearrange("p (a b c d) -> p a b c d",
                                        a=2, b=2, c=64)
                    for bb in range(2):
                        for dd in range(2):
                            nc.vector.tensor_tensor(
                                out=aa_v[:, :, bb, :, dd],
                                in0=t1g_bv[:, lr0:lr0 + 2, :],
                                in1=rc_v[:, :, bb, :, dd],
                                op=mybir.AluOpType.mult)
                    # ker_scaled = es * aa -> bf16 directly into ker_s
                    # (cols (h, u, pp, x, v)); split per (q, x, h) so every
                    # AP is partition + 2 free dims in matching (u, v) order
                    pp0 = blk * 4 + nb2 * 2
                    es_v = es.rearrange("p (q x h u v) -> p q x h u v",
                                        q=2, x=2, h=2, u=32)
                    aa_v5 = aa.rearrange("p (q x h u v) -> p q x h u v",
                                         q=2, x=2, h=2, u=32)
                    for q in range(2):
                        for xx in range(2):
                            for h in range(2):
                                nc.vector.tensor_tensor(
                                    out=ker_v[:, h, :, xx, :, pp0 + q],
                                    in0=es_v[:, q, xx, h],
                                    in1=aa_v5[:, q, xx, h],
                                    op=mybir.AluOpType.mult)

        # ---- scatter ker into banded S (DRAM -> DRAM) ----
        # S row (group-relative) = 36*(dy - dy0g) + u + dx; bands for fixed
        # (dy,dx,h,u) are 128 contiguous elements (pp,x,v).
        for dy in range(5):
            g, dy0g = (0, 0) if dy < 2 else ((1, 2) if dy < 4 else (2, 4))
            s_t = [sA_d, sB_d, sC_d][g]
            grows = 72 if g < 2 else 36
            for dx in range(5):
                tau = dy * 5 + dx
                for h in range(2):
                    base = (h * grows + 36 * (dy - dy0g) + dx) * SCOL
                    # dst row = u, col = u*128 + (x,v,pp): 256B runs
                    dst = bass.AP(tensor=s_t, offset=base,
                                  ap=[[SCOL + 128, 32], [1, 128]])
                    eng = nc.sync if (tau + h) % 2 == 0 else nc.scalar
                    eng.dma_start(
                        dst, ker_s[tau:tau + 1, h * 4096:(h + 1) * 4096])

        # read back
        mid_pool.__exit__(None, None, None)
        s3big = ctx.enter_context(tc.tile_pool(name="s3big", bufs=1))
        s_sbA = [s3big.tile([72, SCOL], BF16, name=f"sA_{h}") for h in range(2)]
        s_sbB = [s3big.tile([72, SCOL], BF16, name=f"sB_{h}") for h in range(2)]
        s_sbC = [s3big.tile([36, SCOL], BF16, name=f"sC_{h}") for h in range(2)]
        sA_v_d = sA_d[:].rearrange("(r c) -> r c", c=SCOL)
        sB_v_d = sB_d[:].rearrange("(r c) -> r c", c=SCOL)
        sC_v_d = sC_d[:].rearrange("(r c) -> r c", c=SCOL)
        for h in range(2):
            eng = nc.sync if h == 0 else nc.scalar
            eng.dma_start(s_sbA[h], sA_v_d[h * 72:(h + 1) * 72])
            eng.dma_start(s_sbB[h], sB_v_d[h * 72:(h + 1) * 72])
            eng.dma_start(s_sbC[h], sC_v_d[h * 36:(h + 1) * 36])
        # lo-res-cols gate map replicated on all partitions (contiguous DMA);
        # the x2 column upsample happens via step-0 dims at the consumer
        g_hi2 = s3big.tile([P, NPAIR * 64], F32)
        nc.sync.dma_start(g_hi2, bass.AP(tensor=g_scr, offset=0,
                                         ap=[[0, P], [1, NPAIR * 64]]))

        # ---- stage 3: carafe + combine ----
        with tc.tile_pool(name="work3", bufs=4) as work3, \
             tc.tile_pool(name="outp", bufs=2) as outp, \
             tc.tile_pool(name="ps_c", bufs=4, space="PSUM") as ps_c:

            # S cols are (u, x, v, pp)
            sA_v = [s_sbA[h].rearrange("k (u x v q) -> k u x v q",
                                       u=32, x=2, v=2) for h in range(2)]
            sB_v = [s_sbB[h].rearrange("k (u x v q) -> k u x v q",
                                       u=32, x=2, v=2) for h in range(2)]
            sC_v = [s_sbC[h].rearrange("k (u x v q) -> k u x v q",
                                       u=32, x=2, v=2) for h in range(2)]
            for grp in range(8):  # groups of 4 pairs
                ost = [outp.tile([P, 8, OC], F32, name=f"ost{a}",
                                 tag=f"ost{a}") for a in range(NA)]
                for lp in range(4):
                    pp = grp * 4 + lp
                    g_up = (g_hi2[:, pp * 64:pp * 64 + 64].unsqueeze(2)
                            .broadcast_to([P, 64, 2]))
                    for a in range(NA):
                        # psum free: (h, u, x, v)
                        ps = ps_c.tile([P, 2, 32, 2, 2], F32)
                        for h in range(2):
                            out_ap = ps[:, h]
                            nc.tensor.matmul(out_ap,
                                             dg_pair[h][:, pp, a, :],
                                             sA_v[h][:, :, :, :, pp],
                                             start=True, stop=False)
                            nc.tensor.matmul(out_ap,
                                             dg_pair[h][:, pp + 2, a, :],
                                             sB_v[h][:, :, :, :, pp],
                                             start=False, stop=False)
                            nc.tensor.matmul(out_ap,
                                             dg_pair[h][0:36, pp + 4, a, :],
                                             sC_v[h][:, :, :, :, pp],
                                             start=False, stop=True)
                        for xx in range(2):
                            tm = work3.tile([P, OC], F32, tag="tm")
                            en_r = en_sb[:, a, 2 * pp + 1 + xx, 1:129]
                            nc.vector.tensor_tensor(
                                out=tm.rearrange("p (w v) -> p w v", v=2),
                                in0=en_r.rearrange("p (w v) -> p w v", v=2),
                                in1=g_up, op=mybir.AluOpType.mult)
                            for h in range(2):
                                nc.vector.tensor_tensor(
                                    out=ost[a][:, 2 * lp + xx,
                                               64 * h:64 * h + 64]
                                        .rearrange("p (u v) -> p u v", v=2),
                                    in0=tm[:, 64 * h:64 * h + 64]
                                        .rearrange("p (u v) -> p u v", v=2),
                                    in1=ps[:, h, :, xx, :],
                                    op=mybir.AluOpType.add)
                for a in range(NA):
                    nc.sync.dma_start(out_d[a, :, grp * 8:grp * 8 + 8, :],
                                      ost[a])


def _get_nc():
    if "nc" not in _NC_CACHE:
        _NC_CACHE["nc"] = _build_nc()
    return _NC_CACHE["nc"]


def _shard_inputs(en, de, w_gate, b_gate, w_cen, b_cen, w_cde, w_ce, b_ce):
    en = np.asarray(en, np.float32)
    de = np.asarray(de, np.float32)
    wgT = np.asarray(w_gate, np.float32).reshape(C)          # (256,)
    wcen = np.asarray(w_cen, np.float32).reshape(EMB, C)
    wcde = np.asarray(w_cde, np.float32).reshape(EMB, C)
    wce = np.asarray(w_ce, np.float32)                       # (25,64,3,3)
    b_gate = np.asarray(b_gate, np.float32).reshape(1)
    b_cen = np.asarray(b_cen, np.float32).reshape(EMB)
    b_ce = np.asarray(b_ce, np.float32).reshape(25)

    wg_h = wgT.reshape(NA, P).T.copy()                        # [128,2]
    wcen_h = np.zeros((P, NA, P), np.float32)
    wcde_h = np.zeros((P, NA, P), np.float32)
    for a in range(NA):
        wt = wcen[:, a * P:(a + 1) * P].T                     # [128,64]
        wcen_h[:, a, 0:EMB] = wt
        wcen_h[:, a, EMB:P] = wt
        wt2 = wcde[:, a * P:(a + 1) * P].T
        wcde_h[:, a, 0:EMB] = wt2
        wcde_h[:, a, EMB:P] = wt2
    bcen_h = np.concatenate([b_cen, b_cen]).reshape(P, 1).astype(np.float32)

    wce2_h = np.zeros((P, 3, 25), np.float32)
    wce1_h = np.zeros((EMB, 3, 25), np.float32)
    for dx in range(3):
        wce2_h[0:EMB, dx, :] = wce[:, :, 0, dx].T
        wce2_h[EMB:P, dx, :] = wce[:, :, 1, dx].T
        wce1_h[:, dx, :] = wce[:, :, 2, dx].T
    wce2_h = wce2_h.astype(ml_dtypes.bfloat16)
    wce1_h = wce1_h.astype(ml_dtypes.bfloat16)

    base = {
        "wg": wg_h, "wgb": b_gate.reshape(1, 1),
        "wgbn": (-b_gate).reshape(1, 1).astype(np.float32),
        "wcen": wcen_h, "bcen": bcen_h, "wcde": wcde_h,
        "wce2": wce2_h, "wce1": wce1_h,
        "bce": b_ce.reshape(25, 1),
        "ident": np.eye(P, dtype=np.float32).astype(ml_dtypes.bfloat16),
        "s_zeroA": np.zeros(144 * SCOL, dtype=ml_dtypes.bfloat16),
        "s_zeroB": np.zeros(144 * SCOL, dtype=ml_dtypes.bfloat16),
        "s_zeroC": np.zeros(72 * SCOL, dtype=ml_dtypes.bfloat16),
    }

    in_maps = []
    for core in range(8):
        b, half = divmod(core, 2)
        r0 = half * 32
        # en shard rows 2r0-1 .. 2r0+65, cols padded +-1
        en_sh = np.zeros((C, EN_R, EN_C), np.float32)
        lo = 2 * r0 - 1
        hi = 2 * r0 + 65
        slo, shi = max(0, lo), min(2 * H, hi)
        en_sh[:, slo - lo:shi - lo, 1:129] = en[b, :, slo:shi, :]
        # de shard rows r0-2 .. r0+34, cols padded +-2
        de_sh = np.zeros((C, LO_R, LO_C), np.float32)
        dlo, dhi = r0 - 2, r0 + 34
        sdlo, sdhi = max(0, dlo), min(H, dhi)
        de_sh[:, sdlo - dlo:sdhi - dlo, 2:66] = de[b, :, sdlo:sdhi, :]
        m = dict(base)
        m["en"] = en_sh.reshape(NA, P, EN_R, EN_C)
        m["de"] = de_sh.reshape(NA, P, LO_R, LO_C)
        in_maps.append(m)
    return in_maps


def kernel(**inputs):
    from concourse.bass_utils import run_bass_kernel_spmd
    nc = _get_nc()
    in_maps = _shard_inputs(**inputs)
    res = run_bass_kernel_spmd(nc, in_maps, core_ids=list(range(8)))
    out = np.zeros((B, C, 2 * H, 2 * W), np.float32)
    for core in range(8):
        b, half = divmod(core, 2)
        o = res.results[core]["out"]                # [2,128,64,128]
        out[b, :, 64 * half:64 * half + 64, :] = o.reshape(C, OR_, OC)
    return out
